# revision 11
# baseline (speedup 1.0000x reference)
"""Trainium2 Bass kernel for the DIN-style pairwise-interaction attention module.

Math (per batch b):
  h = x @ ln_w + ln_b                                  [L, H]
  pre[i,j,a] = a_j + c_i + cross_ij + b1[a]            (w1a/w1b/w1c split of w1)
  score[i,j] = sum_a w2[a]*leaky_relu(pre) + b2, causal-masked (j<=i)
  out = score @ h

Strategy: data-parallel over B=32 across 8 cores (4 batches/core).
Per (b, channel): psum[j,i] = s_a * pre via two accumulating matmuls:
  MM1 (K=64, pure cross): lhsT = hT, rhs_a = s_a*w1c_a . hT
  MM2 (K=38): lhsT=[aT'; ones], rhs = [one-hot | c-row] -> a_j + (c_i + b1)
  (c-row = per-batch flattened cT' injected via DRAM-bounce DMA into the
   one-hot tile's spare row.)
PE row-group packing: channels 0-17 use PE rows 0-63 (lhsT = hh[0:64]),
channels 18-35 use rows 64-127 (lhsT = hh[64:128] = same hT) -> two matmuls
run concurrently in disjoint row-groups, halving effective PE time.
All channels scaled by s_a=|w2[a]| (lrelu positive homogeneity); channels
permuted pos-first and the w2<0 block is SUBTRACTED after separate fold-trees
(HW Lrelu has fixed 0.01 slope; its alpha operand is ignored).
Causal split j in [0,128),[128,200) limits i-extent to 200/72.
"""

import os
import sys

import numpy as np

if "/opt/trn_rl_repo" not in sys.path:
    sys.path.insert(0, "/opt/trn_rl_repo")

import ml_dtypes  # noqa: E402

BF = ml_dtypes.bfloat16

_LRELU1 = None


def _get_lrelu1():
    """Register (once) a custom single-src DVE leaky-relu: out = max(s0*x, x).

    Lets the Vector engine act as a second activation lane beside the
    Scalar engine (PSUM f32 in, bf16 out, one read port)."""
    global _LRELU1
    if _LRELU1 is not None:
        return _LRELU1
    import concourse.dve_ops as dve_ops
    from concourse.dve_spec import Spec, Src0, C0, maxx, lower, _has_src1
    from concourse.dve_uop import DveOpSpec

    name = "LRELU1_ANT_K"
    spec = Spec(
        body=maxx(Src0 * C0, Src0),
        reference=lambda in0, in1, s0, s1, imm2: np.maximum(
            in0.astype(np.float32) * s0, in0.astype(np.float32)
        ),
    )
    shas = {}
    for ver in ("v3", "v4"):
        uops = lower(spec, ver=ver)
        tmp = DveOpSpec(name=name, opcode=1, uops=uops, rd1_en=_has_src1(spec))
        shas[ver] = tmp.sha(ver)
    op = dve_ops.DveOp(name, spec, subdim=False, uops_sha=shas)
    dve_ops.OPS.append(op)
    dve_ops.CUSTOM_DVE_SPECS[name] = spec
    dve_ops._SUB_OPCODE_FOR_NAME[name] = max(dve_ops._SUB_OPCODE_FOR_NAME.values()) + 1
    _LRELU1 = op
    return op

B, L, D = 32, 200, 64
H, A = 64, 36
NEG_SLOPE = 0.01
NCORES = 8
BPC = B // NCORES  # batches per core
J0, J1 = 128, 72
AH = A // 2  # channels per PE-row-half (18)
# activation lane assignment: these waves run on Vector (custom lrelu),
# the rest on Scalar — the two engines are the kernel's twin bottleneck
JB0_V_WAVES = frozenset({5, 7})
JB1_V_WAVES = frozenset()


def _host_prep(ln_w, ln_b, w1, b1, w2, b2):
    """Permute channels (w2>=0 first) and fold |w2| scales into weights."""
    w1a, w1b, w1c = w1[:H], w1[H : 2 * H], w1[2 * H :]
    pos = w2 >= 0
    perm = np.concatenate([np.where(pos)[0], np.where(~pos)[0]])
    npos = int(pos.sum())
    w1a, w1b, w1c = w1a[:, perm], w1b[:, perm], w1c[:, perm]
    b1p, w2p = b1[perm], w2[perm]
    s = np.abs(w2p).astype(np.float32)  # sign handled by subtract-fold

    AH_ = A // 2
    w1cs = (w1c * s).astype(np.float32)
    scl = np.zeros((128, AH_ * 200), np.float32)  # expanded: block c = scale col
    for c in range(AH_):
        scl[0:64, c * 200 : (c + 1) * 200] = w1cs[:, c : c + 1]
        scl[64:128, c * 200 : (c + 1) * 200] = w1cs[:, AH_ + c : AH_ + c + 1]
    scl = scl.astype(BF)
    # compose through the layernorm: aT' = w1as^T @ [hT; 1] = waComp^T @ [xT; 1]
    w1a_s, w1b_s = w1a * s, w1b * s
    w1as = np.zeros((D + 1, A + 1), np.float32)
    w1as[0:D, 0:A] = ln_w @ w1a_s
    w1as[D, 0:A] = ln_b @ w1a_s
    w1as[D, A] = 1.0  # ones output col (reads xT's ones row)
    w1as = w1as.astype(BF)
    w1bs = np.zeros((D + 1, A), np.float32)
    w1bs[0:D] = ln_w @ w1b_s
    w1bs[D] = ln_b @ w1b_s + b1p * s  # b1 folded in
    w1bs = w1bs.astype(BF)
    lnw = np.vstack([ln_w, ln_b[None, :]]).astype(BF)  # [D+1, H]
    # one-hot tile [128, AH*L]: row c selects aT' row c (top); row 64+AH+c
    # selects row AH+c (bottom); rows 36/100 are the per-batch c-row targets
    oh = np.zeros((128, AH * L), dtype=np.float32)
    for c in range(AH):
        oh[c, c * L : (c + 1) * L] = 1.0
        oh[64 + AH + c, c * L : (c + 1) * L] = 1.0
    oh = oh.astype(BF)
    idm = np.eye(128, dtype=BF)
    m0 = (np.arange(L)[None, :] >= np.arange(J0)[:, None]).astype(BF)
    m1 = (np.arange(J1)[None, :] >= np.arange(J1)[:, None]).astype(BF)
    return (
        dict(scl=scl, w1as=w1as, w1bs=w1bs, lnw=lnw, oh=oh, idm=idm, m0=m0, m1=m1),
        npos,
        float(b2),
    )


def _build(npos, b2):
    import concourse.bacc as bacc
    import concourse.tile as tile
    from concourse import mybir

    f32, bf16 = mybir.dt.float32, mybir.dt.bfloat16
    LR = mybir.ActivationFunctionType.Lrelu
    lrelu1 = _get_lrelu1()

    nc = bacc.Bacc("TRN2", target_bir_lowering=False, debug=False)
    x_d = nc.dram_tensor("x", [BPC, L, D], bf16, kind="ExternalInput")
    out_d = nc.dram_tensor("out", [BPC, L, H], f32, kind="ExternalOutput")
    scl_d = nc.dram_tensor("scl", [128, AH * L], bf16, kind="ExternalInput")
    w1as_d = nc.dram_tensor("w1as", [D + 1, A + 1], bf16, kind="ExternalInput")
    w1bs_d = nc.dram_tensor("w1bs", [D + 1, A], bf16, kind="ExternalInput")
    lnw_d = nc.dram_tensor("lnw", [D + 1, H], bf16, kind="ExternalInput")
    oh_d = nc.dram_tensor("oh", [128, AH * L], bf16, kind="ExternalInput")
    idm_d = nc.dram_tensor("idm", [128, 128], bf16, kind="ExternalInput")
    m0_d = nc.dram_tensor("m0", [J0, L], bf16, kind="ExternalInput")
    m1_d = nc.dram_tensor("m1", [J1, J1], bf16, kind="ExternalInput")


    with tile.TileContext(nc) as tc:
        with (
            tc.tile_pool(name="consts", bufs=1) as cp,
            tc.tile_pool(name="prep", bufs=1) as pp,
            tc.tile_pool(name="work", bufs=2) as wp,
            tc.tile_pool(name="psw", bufs=3, space="PSUM") as psw,
            tc.tile_pool(name="psp", bufs=2, space="PSUM") as psp,
        ):
            idm = cp.tile([128, 128], bf16)
            nc.sync.dma_start(idm[:], idm_d[:])
            lnw = cp.tile([D + 1, H], bf16)
            nc.sync.dma_start(lnw[:], lnw_d[:])
            w1as = cp.tile([D + 1, A + 1], bf16)
            nc.sync.dma_start(w1as[:], w1as_d[:])
            w1bs = cp.tile([D + 1, A], bf16)
            nc.sync.dma_start(w1bs[:], w1bs_d[:])
            scl = cp.tile([128, AH * L], bf16)
            nc.scalar.dma_start(scl[:], scl_d[:])
            m0 = cp.tile([J0, L], bf16)
            nc.scalar.dma_start(m0[:], m0_d[:])
            m1 = cp.tile([J1, J1], bf16)
            nc.scalar.dma_start(m1[:], m1_d[:])
            # per-batch one-hot tiles: crow rows 36/100 rewritten per batch
            OH2 = []
            for k in range(BPC):
                t = cp.tile([128, AH * L], bf16, tag=f"oh2_{k}")
                nc.scalar.dma_start(t[:], oh_d[:])
                OH2.append(t)

            def build_rhs(hh_):
                rhs = wp.tile([128, AH * L], bf16, tag="rhs")
                hv = (
                    hh_[:, :]
                    .rearrange("p (o x) -> p o x", o=1)
                    .broadcast_to([128, AH, L])
                )
                sv = scl[:, :].rearrange("p (c x) -> p c x", x=L)
                rv = rhs[:, :].rearrange("p (c x) -> p c x", x=L)
                half = AH // 2
                nc.vector.tensor_mul(
                    rv[:, 0:half], hv[:, 0:half], sv[:, 0:half]
                )
                nc.gpsimd.tensor_mul(
                    rv[:, half:AH], hv[:, half:AH], sv[:, half:AH]
                )
                return rhs

            # ---------- phase 1: per-batch prep ----------
            HH, ATS, H0, H1 = [], [], [], []
            RHS = {}
            for bi in range(BPC):
                x0 = wp.tile([128, D], bf16, tag="x0")
                nc.sync.dma_start(x0[:], x_d[bi, 0:128, :])
                x1 = wp.tile([J1, D], bf16, tag="x1")
                nc.sync.dma_start(x1[:], x_d[bi, 128:L, :])
                xT = wp.tile([D + 1, L], bf16, tag="xT")
                pt0 = psp.tile([D, 128], bf16, tag="pp")
                nc.tensor.transpose(pt0[:], x0[:], idm[:, :])
                nc.vector.tensor_copy(xT[0:D, 0:128], pt0[:])
                pt1 = psp.tile([D, J1], bf16, tag="pp")
                nc.tensor.transpose(pt1[:], x1[:], idm[0:J1, 0:J1])
                nc.vector.tensor_copy(xT[0:D, 128:L], pt1[:])
                nc.vector.memset(xT[D : D + 1, :], 1.0)

                ph = psp.tile([H, L], f32, tag="pp")
                nc.tensor.matmul(ph[:], lnw[:], xT[:], start=True, stop=True)
                hh = pp.tile([128, L], bf16, tag=f"hh{bi}")  # [hT; hT]
                nc.scalar.copy(hh[0:H, :], ph[:])
                nc.scalar.copy(hh[H:128, :], ph[:])

                ph0 = psp.tile([128, H], f32, tag="pp")
                nc.tensor.matmul(ph0[:], xT[:, 0:128], lnw[:], start=True, stop=True)
                h0 = pp.tile([128, H], bf16, tag=f"h0{bi}")
                nc.vector.tensor_copy(h0[:], ph0[:])
                ph1 = psp.tile([J1, H], f32, tag="pp")
                nc.tensor.matmul(ph1[:], xT[:, 128:L], lnw[:], start=True, stop=True)
                h1 = pp.tile([J1, H], bf16, tag=f"h1{bi}")
                nc.vector.tensor_copy(h1[:], ph1[:])

                # aTs: rows 0-36 = [aT'; ones], rows 64-100 = same (bottom copy)
                pa = psp.tile([A + 1, L], f32, tag="pp")
                nc.tensor.matmul(pa[:], w1as[:], xT[:], start=True, stop=True)
                aTs = pp.tile([128, L], bf16, tag=f"aTs{bi}")
                nc.scalar.copy(aTs[0 : A + 1, :], pa[:])
                nc.scalar.copy(aTs[64 : 64 + A + 1, :], pa[:])

                # cT' (+b1) -> flatten into one-hot tile rows 36 / 100
                pc = psp.tile([A, L], f32, tag="pp")
                nc.tensor.matmul(pc[:], w1bs[:], xT[:], start=True, stop=True)
                ctb = wp.tile([A, L], bf16, tag="ctb")
                nc.vector.tensor_copy(ctb[:], pc[:])
                oh2 = OH2[bi]
                nc.gpsimd.dma_start(
                    oh2[A : A + 1, :].rearrange("p (c x) -> p c x", x=L)[0:1],
                    ctb[0:AH, :],
                )
                nc.gpsimd.dma_start(
                    oh2[64 + A : 64 + A + 1, :].rearrange("p (c x) -> p c x", x=L)[0:1],
                    ctb[AH:A, :],
                )

                HH.append(hh)
                ATS.append(aTs)
                H0.append(h0)
                H1.append(h1)
                if bi < 2:
                    RHS[bi] = build_rhs(hh)

            # ---------- phase 2: packed channel waves, folds, output ----------
            for bi in range(BPC):
                hh, aTs = HH[bi], ATS[bi]
                h0, h1 = H0[bi], H1[bi]
                oh2 = OH2[bi]
                rhs = RHS.pop(bi)

                r0 = wp.tile([J0, A * L], bf16, tag="r0")
                r1 = wp.tile([J1, A * J1], bf16, tag="r1")

                # jb0: 9 waves; wave t = top pair (2t,2t+1) + bottom pair (+18)
                for t in range(AH // 2):
                    pw = psw.tile([J0, 1024], f32, tag="pw")
                    cols = slice(2 * t * L, (2 * t + 2) * L)
                    nc.tensor.matmul(
                        pw[:, 0:400], hh[0:H, 0:J0], rhs[0:H, cols],
                        start=True, stop=False,
                    )
                    nc.tensor.matmul(
                        pw[:, 0:400], aTs[0 : A + 1, 0:J0], oh2[0 : A + 1, cols],
                        start=False, stop=True,
                    )
                    nc.tensor.matmul(
                        pw[:, 512:912], hh[H:128, 0:J0], rhs[H:128, cols],
                        start=True, stop=False,
                    )
                    nc.tensor.matmul(
                        pw[:, 512:912],
                        aTs[64 : 64 + A + 1, 0:J0],
                        oh2[64 : 64 + A + 1, cols],
                        start=False, stop=True,
                    )
                    r0v = r0[:, :].rearrange("p (g y) -> p g y", y=AH * L)[
                        :, :, 2 * t * L : (2 * t + 2) * L
                    ]
                    pwv = pw[:, :].rearrange("p (g y) -> p g y", y=512)[:, :, 0:400]
                    if t in JB0_V_WAVES:
                        nc.vector._custom_dve(
                            lrelu1, out=r0v, in0=pwv, s0=NEG_SLOPE
                        )
                    else:
                        nc.scalar.activation(r0v, pwv, LR, alpha=NEG_SLOPE)

                # jb1: 5 waves of up-to-4 channels per half, 128-padded slots
                rhv_t = rhs[0:H, :].rearrange("p (c x) -> p c x", x=L)
                rhv_b = rhs[H:128, :].rearrange("p (c x) -> p c x", x=L)
                ohv_t = oh2[0 : A + 1, :].rearrange("p (c x) -> p c x", x=L)
                ohv_b = oh2[64 : 64 + A + 1, :].rearrange("p (c x) -> p c x", x=L)
                for w in range((AH + 3) // 4):
                    c0 = 4 * w
                    gn = min(4, AH - c0)
                    pz = psw.tile([J1, 1024], f32, tag="pw")
                    pzv = pz[:, :].rearrange("p (g x) -> p g x", x=128)
                    nc.tensor.matmul(
                        pzv[:, 0:gn, 0:J1],
                        hh[0:H, 128:L],
                        rhv_t[:, c0 : c0 + gn, 128:L],
                        start=True, stop=False,
                    )
                    nc.tensor.matmul(
                        pzv[:, 0:gn, 0:J1],
                        aTs[0 : A + 1, 128:L],
                        ohv_t[:, c0 : c0 + gn, 128:L],
                        start=False, stop=True,
                    )
                    nc.tensor.matmul(
                        pzv[:, 4 : 4 + gn, 0:J1],
                        hh[H:128, 128:L],
                        rhv_b[:, c0 : c0 + gn, 128:L],
                        start=True, stop=False,
                    )
                    nc.tensor.matmul(
                        pzv[:, 4 : 4 + gn, 0:J1],
                        aTs[64 : 64 + A + 1, 128:L],
                        ohv_b[:, c0 : c0 + gn, 128:L],
                        start=False, stop=True,
                    )
                    if w in JB1_V_WAVES:
                        for g in range(2):
                            nc.vector._custom_dve(
                                lrelu1,
                                out=r1[:, :].rearrange(
                                    "p (g c x) -> p g c x", g=2, x=J1
                                )[:, g, c0 : c0 + gn, :],
                                in0=pz[:, :].rearrange(
                                    "p (g s x) -> p g s x", g=2, x=128
                                )[:, g, 0:gn, 0:J1],
                                s0=NEG_SLOPE,
                            )
                    else:
                        nc.scalar.activation(
                            r1[:, :]
                            .rearrange("p (g c x) -> p g c x", g=2, x=J1)[
                                :, :, c0 : c0 + gn, :
                            ],
                            pz[:, :]
                            .rearrange("p (g s x) -> p g s x", g=2, x=128)[
                                :, :, 0:gn, 0:J1
                            ],
                            LR,
                            alpha=NEG_SLOPE,
                        )

                if bi + 2 < BPC:
                    RHS[bi + 2] = build_rhs(HH[bi + 2])

                # signed fold: L1 = one big (pos - neg) pass over min(P,N)
                # pairs, leftover blocks merged in chunks, then add-tree;
                # final pass fuses +b2 and causal mask in one STT.
                P, N = npos, A - npos
                m = min(P, N)

                sm0 = wp.tile([J0, L], bf16, tag="sm0")
                sm1 = wp.tile([J1, J1], bf16, tag="sm1")
                for reg, stride, sm, msk in ((r0, L, sm0, m0), (r1, J1, sm1, m1)):
                    if m > 0:
                        nc.vector.tensor_sub(
                            reg[:, 0 : m * stride],
                            reg[:, 0 : m * stride],
                            reg[:, P * stride : (P + m) * stride],
                        )
                        # leftovers: extra pos at [m:P) (add) or extra neg
                        # at [P+m:A) (subtract), merged into d-blocks
                        if P > N:
                            off, sign = N, mybir.AluOpType.add
                        else:
                            off, sign = P + m, mybir.AluOpType.subtract
                        extra = max(P, N) - m
                        o = off
                        while extra > 0:
                            w_ = min(m, extra)
                            nc.vector.tensor_tensor(
                                reg[:, 0 : w_ * stride],
                                reg[:, 0 : w_ * stride],
                                reg[:, o * stride : (o + w_) * stride],
                                sign,
                            )
                            o += w_
                            extra -= w_
                        W = m
                    else:
                        W = A  # all one sign; tree over everything
                    while W > 1:
                        half = W // 2
                        keep = W - half
                        nc.vector.tensor_add(
                            reg[:, 0 : half * stride],
                            reg[:, 0 : half * stride],
                            reg[:, keep * stride : W * stride],
                        )
                        W = keep
                    if m > 0 or P > 0:
                        nc.vector.scalar_tensor_tensor(
                            sm[:],
                            reg[:, 0:stride],
                            b2,
                            msk[:],
                            mybir.AluOpType.add,
                            mybir.AluOpType.mult,
                        )
                    else:  # all-negative: negate, +b2, then mask
                        nc.vector.tensor_scalar(
                            sm[:], reg[:, 0:stride], -1.0, b2,
                            mybir.AluOpType.mult, mybir.AluOpType.add,
                        )
                        nc.vector.tensor_mul(sm[:], sm[:], msk[:])

                # out = masked-score^T @ h
                po1 = psp.tile([128, H], f32, tag="pp")
                nc.tensor.matmul(po1[:], sm0[:, 0:128], h0[:], start=True, stop=True)
                po2 = psp.tile([J1, H], f32, tag="pp")
                nc.tensor.matmul(po2[:], sm0[:, 128:L], h0[:], start=True, stop=False)
                nc.tensor.matmul(po2[:], sm1[:], h1[:], start=False, stop=True)
                o0 = wp.tile([128, H], f32, tag="o0")
                nc.vector.tensor_copy(o0[:], po1[:])
                o1 = wp.tile([J1, H], f32, tag="o1")
                nc.vector.tensor_copy(o1[:], po2[:])
                nc.sync.dma_start(out_d[bi, 0:128, :], o0[:])
                nc.sync.dma_start(out_d[bi, 128:L, :], o1[:])

    if not nc.is_finalized():
        nc.finalize()
    return nc


_CACHE = {}


def kernel(x, ln_w, ln_b, w1, b1, w2, b2):
    from concourse.bass_utils import run_bass_kernel_spmd

    x = np.asarray(x, dtype=np.float32)
    consts, npos, b2f = _host_prep(
        np.asarray(ln_w, np.float32),
        np.asarray(ln_b, np.float32),
        np.asarray(w1, np.float32),
        np.asarray(b1, np.float32),
        np.asarray(w2, np.float32),
        np.asarray(b2, np.float32),
    )
    key = (npos, round(b2f, 9))
    if key not in _CACHE:
        _CACHE[key] = _build(npos, b2f)
    nc = _CACHE[key]

    xb = x.astype(BF)
    in_maps = []
    for c in range(NCORES):
        m = {"x": xb[c * BPC : (c + 1) * BPC]}
        m.update(consts)
        in_maps.append(m)

    trace = bool(int(os.environ.get("KERNEL_TRACE", "0")))
    res = run_bass_kernel_spmd(nc, in_maps, list(range(NCORES)), trace=trace)
    out = np.concatenate([res.results[c]["out"] for c in range(NCORES)], axis=0)
    if trace:
        kernel.last_exec_time_ns = res.exec_time_ns
        kernel.last_results = res
    return out.astype(np.float32)



# revision 12
# speedup vs baseline: 1.0549x; 1.0549x over previous
"""Trainium2 Bass kernel for the DIN-style pairwise-interaction attention module.

Math (per batch b):
  h = x @ ln_w + ln_b                                  [L, H]
  pre[i,j,a] = a_j + c_i + cross_ij + b1[a]            (w1a/w1b/w1c split of w1)
  score[i,j] = sum_a w2[a]*leaky_relu(pre) + b2, causal-masked (j<=i)
  out = score @ h

Strategy: data-parallel over B=32 across 8 cores (4 batches/core).
Per (b, channel): psum[j,i] = s_a * pre via two accumulating matmuls:
  MM1 (K=64, pure cross): lhsT = hT, rhs_a = s_a*w1c_a . hT
  MM2 (K=38): lhsT=[aT'; ones], rhs = [one-hot | c-row] -> a_j + (c_i + b1)
  (c-row = per-batch flattened cT' injected via DRAM-bounce DMA into the
   one-hot tile's spare row.)
PE row-group packing: channels 0-17 use PE rows 0-63 (lhsT = hh[0:64]),
channels 18-35 use rows 64-127 (lhsT = hh[64:128] = same hT) -> two matmuls
run concurrently in disjoint row-groups, halving effective PE time.
All channels scaled by s_a=|w2[a]| (lrelu positive homogeneity); channels
permuted pos-first and the w2<0 block is SUBTRACTED after separate fold-trees
(HW Lrelu has fixed 0.01 slope; its alpha operand is ignored).
Causal split j in [0,128),[128,200) limits i-extent to 200/72.
"""

import os
import sys

import numpy as np

if "/opt/trn_rl_repo" not in sys.path:
    sys.path.insert(0, "/opt/trn_rl_repo")

import ml_dtypes  # noqa: E402

BF = ml_dtypes.bfloat16

_LRELU1 = None


def _get_lrelu1():
    """Register (once) a custom single-src DVE leaky-relu: out = max(s0*x, x).

    Lets the Vector engine act as a second activation lane beside the
    Scalar engine (PSUM f32 in, bf16 out, one read port)."""
    global _LRELU1
    if _LRELU1 is not None:
        return _LRELU1
    import concourse.dve_ops as dve_ops
    from concourse.dve_spec import Spec, Src0, C0, maxx, lower, _has_src1
    from concourse.dve_uop import DveOpSpec

    name = "LRELU1_ANT_K"
    spec = Spec(
        body=maxx(Src0 * C0, Src0),
        reference=lambda in0, in1, s0, s1, imm2: np.maximum(
            in0.astype(np.float32) * s0, in0.astype(np.float32)
        ),
    )
    shas = {}
    for ver in ("v3", "v4"):
        uops = lower(spec, ver=ver)
        tmp = DveOpSpec(name=name, opcode=1, uops=uops, rd1_en=_has_src1(spec))
        shas[ver] = tmp.sha(ver)
    op = dve_ops.DveOp(name, spec, subdim=False, uops_sha=shas)
    dve_ops.OPS.append(op)
    dve_ops.CUSTOM_DVE_SPECS[name] = spec
    dve_ops._SUB_OPCODE_FOR_NAME[name] = max(dve_ops._SUB_OPCODE_FOR_NAME.values()) + 1
    _LRELU1 = op
    return op

B, L, D = 32, 200, 64
H, A = 64, 36
NEG_SLOPE = 0.01
NCORES = 8
BPC = B // NCORES  # batches per core
J0, J1 = 128, 72
AH = A // 2  # channels per PE-row-half (18)
# activation lane assignment: these waves run on Vector (custom lrelu),
# the rest on Scalar — the two engines are the kernel's twin bottleneck
JB0_V_WAVES = frozenset({5, 7})
JB1_V_WAVES = frozenset()


def _host_prep(ln_w, ln_b, w1, b1, w2, b2):
    """Permute channels (w2>=0 first) and fold |w2| scales into weights."""
    w1a, w1b, w1c = w1[:H], w1[H : 2 * H], w1[2 * H :]
    pos = w2 >= 0
    perm = np.concatenate([np.where(pos)[0], np.where(~pos)[0]])
    npos = int(pos.sum())
    w1a, w1b, w1c = w1a[:, perm], w1b[:, perm], w1c[:, perm]
    b1p, w2p = b1[perm], w2[perm]
    s = np.abs(w2p).astype(np.float32)  # sign handled by subtract-fold

    AH_ = A // 2
    w1cs = (w1c * s).astype(np.float32)
    scl = np.zeros((128, AH_ * 200), np.float32)  # expanded: block c = scale col
    for c in range(AH_):
        scl[0:64, c * 200 : (c + 1) * 200] = w1cs[:, c : c + 1]
        scl[64:128, c * 200 : (c + 1) * 200] = w1cs[:, AH_ + c : AH_ + c + 1]
    scl = scl.astype(BF)
    # compose through the layernorm: aT' = w1as^T @ [hT; 1] = waComp^T @ [xT; 1]
    w1a_s, w1b_s = w1a * s, w1b * s
    w1as = np.zeros((D + 1, A + 1), np.float32)
    w1as[0:D, 0:A] = ln_w @ w1a_s
    w1as[D, 0:A] = ln_b @ w1a_s
    w1as[D, A] = 1.0  # ones output col (reads xT's ones row)
    w1as = w1as.astype(BF)
    w1bs = np.zeros((D + 1, A), np.float32)
    w1bs[0:D] = ln_w @ w1b_s
    w1bs[D] = ln_b @ w1b_s + b1p * s  # b1 folded in
    w1bs = w1bs.astype(BF)
    lnw = np.vstack([ln_w, ln_b[None, :]]).astype(BF)  # [D+1, H]
    # one-hot tile [128, AH*L]: row c selects aT' row c (top); row 64+AH+c
    # selects row AH+c (bottom); rows 36/100 are the per-batch c-row targets
    oh = np.zeros((128, AH * L), dtype=np.float32)
    for c in range(AH):
        oh[c, c * L : (c + 1) * L] = 1.0
        oh[64 + AH + c, c * L : (c + 1) * L] = 1.0
    oh = oh.astype(BF)
    idm = np.eye(128, dtype=BF)
    m0 = (np.arange(L)[None, :] >= np.arange(J0)[:, None]).astype(BF)
    m1 = (np.arange(J1)[None, :] >= np.arange(J1)[:, None]).astype(BF)
    return (
        dict(scl=scl, w1as=w1as, w1bs=w1bs, lnw=lnw, oh=oh, idm=idm, m0=m0, m1=m1),
        npos,
        float(b2),
    )


def _build(npos, b2):
    import concourse.bacc as bacc
    import concourse.tile as tile
    from concourse import mybir

    f32, bf16 = mybir.dt.float32, mybir.dt.bfloat16
    LR = mybir.ActivationFunctionType.Lrelu
    lrelu1 = _get_lrelu1()

    nc = bacc.Bacc("TRN2", target_bir_lowering=False, debug=False)
    x_d = nc.dram_tensor("x", [BPC, L, D], bf16, kind="ExternalInput")
    out_d = nc.dram_tensor("out", [BPC, L, H], f32, kind="ExternalOutput")
    scl_d = nc.dram_tensor("scl", [128, AH * L], bf16, kind="ExternalInput")
    w1as_d = nc.dram_tensor("w1as", [D + 1, A + 1], bf16, kind="ExternalInput")
    w1bs_d = nc.dram_tensor("w1bs", [D + 1, A], bf16, kind="ExternalInput")
    lnw_d = nc.dram_tensor("lnw", [D + 1, H], bf16, kind="ExternalInput")
    oh_d = nc.dram_tensor("oh", [128, AH * L], bf16, kind="ExternalInput")
    idm_d = nc.dram_tensor("idm", [128, 128], bf16, kind="ExternalInput")
    m0_d = nc.dram_tensor("m0", [J0, L], bf16, kind="ExternalInput")
    m1_d = nc.dram_tensor("m1", [J1, J1], bf16, kind="ExternalInput")


    with tile.TileContext(nc) as tc:
        with (
            tc.tile_pool(name="consts", bufs=1) as cp,
            tc.tile_pool(name="prep", bufs=1) as pp,
            tc.tile_pool(name="work", bufs=2) as wp,
            tc.tile_pool(name="psw", bufs=3, space="PSUM") as psw,
            tc.tile_pool(name="psp", bufs=2, space="PSUM") as psp,
        ):
            idm = cp.tile([128, 128], bf16)
            nc.sync.dma_start(idm[:], idm_d[:])
            lnw = cp.tile([D + 1, H], bf16)
            nc.sync.dma_start(lnw[:], lnw_d[:])
            w1as = cp.tile([D + 1, A + 1], bf16)
            nc.sync.dma_start(w1as[:], w1as_d[:])
            w1bs = cp.tile([D + 1, A], bf16)
            nc.sync.dma_start(w1bs[:], w1bs_d[:])
            scl = cp.tile([128, AH * L], bf16)
            nc.scalar.dma_start(scl[:], scl_d[:])
            m0 = cp.tile([J0, L], bf16)
            nc.scalar.dma_start(m0[:], m0_d[:])
            m1 = cp.tile([J1, J1], bf16)
            nc.scalar.dma_start(m1[:], m1_d[:])
            # per-batch one-hot tiles: crow rows 36/100 rewritten per batch
            OH2 = []
            for k in range(BPC):
                t = cp.tile([128, AH * L], bf16, tag=f"oh2_{k}")
                nc.scalar.dma_start(t[:], oh_d[:])
                OH2.append(t)

            def build_rhs(hh_):
                rhs = wp.tile([128, AH * L], bf16, tag="rhs")
                hv = (
                    hh_[:, :]
                    .rearrange("p (o x) -> p o x", o=1)
                    .broadcast_to([128, AH, L])
                )
                sv = scl[:, :].rearrange("p (c x) -> p c x", x=L)
                rv = rhs[:, :].rearrange("p (c x) -> p c x", x=L)
                nc.vector.tensor_mul(rv[:], hv[:], sv[:])
                return rhs

            # ---------- phase 1: per-batch prep ----------
            HH, ATS, H0, H1 = [], [], [], []
            RHS = {}
            for bi in range(BPC):
                x0 = wp.tile([128, D], bf16, tag="x0")
                nc.sync.dma_start(x0[:], x_d[bi, 0:128, :])
                x1 = wp.tile([J1, D], bf16, tag="x1")
                nc.sync.dma_start(x1[:], x_d[bi, 128:L, :])
                xT = wp.tile([D + 1, L], bf16, tag="xT")
                pt0 = psp.tile([D, 128], bf16, tag="pp")
                nc.tensor.transpose(pt0[:], x0[:], idm[:, :])
                nc.vector.tensor_copy(xT[0:D, 0:128], pt0[:])
                pt1 = psp.tile([D, J1], bf16, tag="pp")
                nc.tensor.transpose(pt1[:], x1[:], idm[0:J1, 0:J1])
                nc.vector.tensor_copy(xT[0:D, 128:L], pt1[:])
                nc.vector.memset(xT[D : D + 1, :], 1.0)

                ph = psp.tile([H, L], f32, tag="pp")
                nc.tensor.matmul(ph[:], lnw[:], xT[:], start=True, stop=True)
                hh = pp.tile([128, L], bf16, tag=f"hh{bi}")  # [hT; hT]
                nc.scalar.copy(hh[0:H, :], ph[:])
                nc.scalar.copy(hh[H:128, :], ph[:])

                ph0 = psp.tile([128, H], f32, tag="pp")
                nc.tensor.matmul(ph0[:], xT[:, 0:128], lnw[:], start=True, stop=True)
                h0 = pp.tile([128, H], bf16, tag=f"h0{bi}")
                nc.vector.tensor_copy(h0[:], ph0[:])
                ph1 = psp.tile([J1, H], f32, tag="pp")
                nc.tensor.matmul(ph1[:], xT[:, 128:L], lnw[:], start=True, stop=True)
                h1 = pp.tile([J1, H], bf16, tag=f"h1{bi}")
                nc.vector.tensor_copy(h1[:], ph1[:])

                # aTs: rows 0-36 = [aT'; ones], rows 64-100 = same (bottom copy)
                pa = psp.tile([A + 1, L], f32, tag="pp")
                nc.tensor.matmul(pa[:], w1as[:], xT[:], start=True, stop=True)
                aTs = pp.tile([128, L], bf16, tag=f"aTs{bi}")
                nc.scalar.copy(aTs[0 : A + 1, :], pa[:])
                nc.scalar.copy(aTs[64 : 64 + A + 1, :], pa[:])

                # cT' (+b1) -> flatten into one-hot tile rows 36 / 100
                pc = psp.tile([A, L], f32, tag="pp")
                nc.tensor.matmul(pc[:], w1bs[:], xT[:], start=True, stop=True)
                ctb = wp.tile([A, L], bf16, tag="ctb")
                nc.vector.tensor_copy(ctb[:], pc[:])
                oh2 = OH2[bi]
                nc.gpsimd.dma_start(
                    oh2[A : A + 1, :].rearrange("p (c x) -> p c x", x=L)[0:1],
                    ctb[0:AH, :],
                )
                nc.gpsimd.dma_start(
                    oh2[64 + A : 64 + A + 1, :].rearrange("p (c x) -> p c x", x=L)[0:1],
                    ctb[AH:A, :],
                )

                HH.append(hh)
                ATS.append(aTs)
                H0.append(h0)
                H1.append(h1)
                if bi < 2:
                    RHS[bi] = build_rhs(hh)

            # ---------- phase 2: packed channel waves, folds, output ----------
            for bi in range(BPC):
                hh, aTs = HH[bi], ATS[bi]
                h0, h1 = H0[bi], H1[bi]
                oh2 = OH2[bi]
                rhs = RHS.pop(bi)

                r0 = wp.tile([J0, A * L], bf16, tag="r0")
                r1 = wp.tile([J1, A * J1], bf16, tag="r1")

                # jb0: 9 waves; wave t = top pair (2t,2t+1) + bottom pair (+18)
                for t in range(AH // 2):
                    pw = psw.tile([J0, 1024], f32, tag="pw")
                    cols = slice(2 * t * L, (2 * t + 2) * L)
                    nc.tensor.matmul(
                        pw[:, 0:400], hh[0:H, 0:J0], rhs[0:H, cols],
                        start=True, stop=False,
                    )
                    nc.tensor.matmul(
                        pw[:, 0:400], aTs[0 : A + 1, 0:J0], oh2[0 : A + 1, cols],
                        start=False, stop=True,
                    )
                    nc.tensor.matmul(
                        pw[:, 512:912], hh[H:128, 0:J0], rhs[H:128, cols],
                        start=True, stop=False,
                    )
                    nc.tensor.matmul(
                        pw[:, 512:912],
                        aTs[64 : 64 + A + 1, 0:J0],
                        oh2[64 : 64 + A + 1, cols],
                        start=False, stop=True,
                    )
                    r0v = r0[:, :].rearrange("p (g y) -> p g y", y=AH * L)[
                        :, :, 2 * t * L : (2 * t + 2) * L
                    ]
                    pwv = pw[:, :].rearrange("p (g y) -> p g y", y=512)[:, :, 0:400]
                    if t in JB0_V_WAVES:
                        nc.vector._custom_dve(
                            lrelu1, out=r0v, in0=pwv, s0=NEG_SLOPE
                        )
                    else:
                        nc.scalar.activation(r0v, pwv, LR, alpha=NEG_SLOPE)

                # jb1: 5 waves of up-to-4 channels per half, 128-padded slots
                rhv_t = rhs[0:H, :].rearrange("p (c x) -> p c x", x=L)
                rhv_b = rhs[H:128, :].rearrange("p (c x) -> p c x", x=L)
                ohv_t = oh2[0 : A + 1, :].rearrange("p (c x) -> p c x", x=L)
                ohv_b = oh2[64 : 64 + A + 1, :].rearrange("p (c x) -> p c x", x=L)
                for w in range((AH + 3) // 4):
                    c0 = 4 * w
                    gn = min(4, AH - c0)
                    pz = psw.tile([J1, 1024], f32, tag="pw")
                    pzv = pz[:, :].rearrange("p (g x) -> p g x", x=128)
                    nc.tensor.matmul(
                        pzv[:, 0:gn, 0:J1],
                        hh[0:H, 128:L],
                        rhv_t[:, c0 : c0 + gn, 128:L],
                        start=True, stop=False,
                    )
                    nc.tensor.matmul(
                        pzv[:, 0:gn, 0:J1],
                        aTs[0 : A + 1, 128:L],
                        ohv_t[:, c0 : c0 + gn, 128:L],
                        start=False, stop=True,
                    )
                    nc.tensor.matmul(
                        pzv[:, 4 : 4 + gn, 0:J1],
                        hh[H:128, 128:L],
                        rhv_b[:, c0 : c0 + gn, 128:L],
                        start=True, stop=False,
                    )
                    nc.tensor.matmul(
                        pzv[:, 4 : 4 + gn, 0:J1],
                        aTs[64 : 64 + A + 1, 128:L],
                        ohv_b[:, c0 : c0 + gn, 128:L],
                        start=False, stop=True,
                    )
                    if w in JB1_V_WAVES:
                        for g in range(2):
                            nc.vector._custom_dve(
                                lrelu1,
                                out=r1[:, :].rearrange(
                                    "p (g c x) -> p g c x", g=2, x=J1
                                )[:, g, c0 : c0 + gn, :],
                                in0=pz[:, :].rearrange(
                                    "p (g s x) -> p g s x", g=2, x=128
                                )[:, g, 0:gn, 0:J1],
                                s0=NEG_SLOPE,
                            )
                    else:
                        nc.scalar.activation(
                            r1[:, :]
                            .rearrange("p (g c x) -> p g c x", g=2, x=J1)[
                                :, :, c0 : c0 + gn, :
                            ],
                            pz[:, :]
                            .rearrange("p (g s x) -> p g s x", g=2, x=128)[
                                :, :, 0:gn, 0:J1
                            ],
                            LR,
                            alpha=NEG_SLOPE,
                        )

                if bi + 2 < BPC:
                    RHS[bi + 2] = build_rhs(HH[bi + 2])

                # signed fold: L1 = one big (pos - neg) pass over min(P,N)
                # pairs, leftover blocks merged in chunks, then add-tree;
                # final pass fuses +b2 and causal mask in one STT.
                P, N = npos, A - npos
                m = min(P, N)

                sm0 = wp.tile([J0, L], bf16, tag="sm0")
                sm1 = wp.tile([J1, J1], bf16, tag="sm1")
                for reg, stride, sm, msk in ((r0, L, sm0, m0), (r1, J1, sm1, m1)):
                    if m > 0:
                        nc.vector.tensor_sub(
                            reg[:, 0 : m * stride],
                            reg[:, 0 : m * stride],
                            reg[:, P * stride : (P + m) * stride],
                        )
                        # leftovers: extra pos at [m:P) (add) or extra neg
                        # at [P+m:A) (subtract), merged into d-blocks
                        if P > N:
                            off, sign = N, mybir.AluOpType.add
                        else:
                            off, sign = P + m, mybir.AluOpType.subtract
                        extra = max(P, N) - m
                        o = off
                        while extra > 0:
                            w_ = min(m, extra)
                            nc.vector.tensor_tensor(
                                reg[:, 0 : w_ * stride],
                                reg[:, 0 : w_ * stride],
                                reg[:, o * stride : (o + w_) * stride],
                                sign,
                            )
                            o += w_
                            extra -= w_
                        W = m
                    else:
                        W = A  # all one sign; tree over everything
                    while W > 1:
                        half = W // 2
                        keep = W - half
                        nc.vector.tensor_add(
                            reg[:, 0 : half * stride],
                            reg[:, 0 : half * stride],
                            reg[:, keep * stride : W * stride],
                        )
                        W = keep
                    if m > 0 or P > 0:
                        nc.vector.scalar_tensor_tensor(
                            sm[:],
                            reg[:, 0:stride],
                            b2,
                            msk[:],
                            mybir.AluOpType.add,
                            mybir.AluOpType.mult,
                        )
                    else:  # all-negative: negate, +b2, then mask
                        nc.vector.tensor_scalar(
                            sm[:], reg[:, 0:stride], -1.0, b2,
                            mybir.AluOpType.mult, mybir.AluOpType.add,
                        )
                        nc.vector.tensor_mul(sm[:], sm[:], msk[:])

                # out = masked-score^T @ h
                po1 = psp.tile([128, H], f32, tag="pp")
                nc.tensor.matmul(po1[:], sm0[:, 0:128], h0[:], start=True, stop=True)
                po2 = psp.tile([J1, H], f32, tag="pp")
                nc.tensor.matmul(po2[:], sm0[:, 128:L], h0[:], start=True, stop=False)
                nc.tensor.matmul(po2[:], sm1[:], h1[:], start=False, stop=True)
                o0 = wp.tile([128, H], f32, tag="o0")
                nc.vector.tensor_copy(o0[:], po1[:])
                o1 = wp.tile([J1, H], f32, tag="o1")
                nc.vector.tensor_copy(o1[:], po2[:])
                nc.sync.dma_start(out_d[bi, 0:128, :], o0[:])
                nc.sync.dma_start(out_d[bi, 128:L, :], o1[:])

    if not nc.is_finalized():
        nc.finalize()
    return nc


_CACHE = {}


def kernel(x, ln_w, ln_b, w1, b1, w2, b2):
    from concourse.bass_utils import run_bass_kernel_spmd

    x = np.asarray(x, dtype=np.float32)
    consts, npos, b2f = _host_prep(
        np.asarray(ln_w, np.float32),
        np.asarray(ln_b, np.float32),
        np.asarray(w1, np.float32),
        np.asarray(b1, np.float32),
        np.asarray(w2, np.float32),
        np.asarray(b2, np.float32),
    )
    key = (npos, round(b2f, 9))
    if key not in _CACHE:
        _CACHE[key] = _build(npos, b2f)
    nc = _CACHE[key]

    xb = x.astype(BF)
    in_maps = []
    for c in range(NCORES):
        m = {"x": xb[c * BPC : (c + 1) * BPC]}
        m.update(consts)
        in_maps.append(m)

    trace = bool(int(os.environ.get("KERNEL_TRACE", "0")))
    res = run_bass_kernel_spmd(nc, in_maps, list(range(NCORES)), trace=trace)
    out = np.concatenate([res.results[c]["out"] for c in range(NCORES)], axis=0)
    if trace:
        kernel.last_exec_time_ns = res.exec_time_ns
        kernel.last_results = res
    return out.astype(np.float32)



# revision 13
# speedup vs baseline: 1.0694x; 1.0137x over previous
"""Trainium2 Bass kernel for the DIN-style pairwise-interaction attention module.

Math (per batch b):
  h = x @ ln_w + ln_b                                  [L, H]
  pre[i,j,a] = a_j + c_i + cross_ij + b1[a]            (w1a/w1b/w1c split of w1)
  score[i,j] = sum_a w2[a]*leaky_relu(pre) + b2, causal-masked (j<=i)
  out = score @ h

Strategy: data-parallel over B=32 across 8 cores (4 batches/core).
Per (b, channel): psum[j,i] = s_a * pre via two accumulating matmuls:
  MM1 (K=64, pure cross): lhsT = hT, rhs_a = s_a*w1c_a . hT
  MM2 (K=38): lhsT=[aT'; ones], rhs = [one-hot | c-row] -> a_j + (c_i + b1)
  (c-row = per-batch flattened cT' injected via DRAM-bounce DMA into the
   one-hot tile's spare row.)
PE row-group packing: channels 0-17 use PE rows 0-63 (lhsT = hh[0:64]),
channels 18-35 use rows 64-127 (lhsT = hh[64:128] = same hT) -> two matmuls
run concurrently in disjoint row-groups, halving effective PE time.
All channels scaled by s_a=|w2[a]| (lrelu positive homogeneity); channels
permuted pos-first and the w2<0 block is SUBTRACTED after separate fold-trees
(HW Lrelu has fixed 0.01 slope; its alpha operand is ignored).
Causal split j in [0,128),[128,200) limits i-extent to 200/72.
"""

import os
import sys

import numpy as np

if "/opt/trn_rl_repo" not in sys.path:
    sys.path.insert(0, "/opt/trn_rl_repo")

import ml_dtypes  # noqa: E402

BF = ml_dtypes.bfloat16

_LRELU1 = None


def _get_lrelu1():
    """Register (once) a custom single-src DVE leaky-relu: out = max(s0*x, x).

    Lets the Vector engine act as a second activation lane beside the
    Scalar engine (PSUM f32 in, bf16 out, one read port)."""
    global _LRELU1
    if _LRELU1 is not None:
        return _LRELU1
    import concourse.dve_ops as dve_ops
    from concourse.dve_spec import Spec, Src0, C0, maxx, lower, _has_src1
    from concourse.dve_uop import DveOpSpec

    name = "LRELU1_ANT_K"
    spec = Spec(
        body=maxx(Src0 * C0, Src0),
        reference=lambda in0, in1, s0, s1, imm2: np.maximum(
            in0.astype(np.float32) * s0, in0.astype(np.float32)
        ),
    )
    shas = {}
    for ver in ("v3", "v4"):
        uops = lower(spec, ver=ver)
        tmp = DveOpSpec(name=name, opcode=1, uops=uops, rd1_en=_has_src1(spec))
        shas[ver] = tmp.sha(ver)
    op = dve_ops.DveOp(name, spec, subdim=False, uops_sha=shas)
    dve_ops.OPS.append(op)
    dve_ops.CUSTOM_DVE_SPECS[name] = spec
    dve_ops._SUB_OPCODE_FOR_NAME[name] = max(dve_ops._SUB_OPCODE_FOR_NAME.values()) + 1
    _LRELU1 = op
    return op

B, L, D = 32, 200, 64
H, A = 64, 36
NEG_SLOPE = 0.01
NCORES = 8
BPC = B // NCORES  # batches per core
J0, J1 = 128, 72
AH = A // 2  # channels per PE-row-half (18)
# activation lane assignment: these waves run on Vector (custom lrelu),
# the rest on Scalar — the two engines are the kernel's twin bottleneck
JB0_V_WAVES = frozenset({5, 7})
JB1_V_WAVES = frozenset()


def _host_prep(ln_w, ln_b, w1, b1, w2, b2):
    """Permute channels (w2>=0 first) and fold |w2| scales into weights."""
    w1a, w1b, w1c = w1[:H], w1[H : 2 * H], w1[2 * H :]
    pos = w2 >= 0
    perm = np.concatenate([np.where(pos)[0], np.where(~pos)[0]])
    npos = int(pos.sum())
    w1a, w1b, w1c = w1a[:, perm], w1b[:, perm], w1c[:, perm]
    b1p, w2p = b1[perm], w2[perm]
    s = np.abs(w2p).astype(np.float32)  # sign handled by subtract-fold

    AH_ = A // 2
    w1cs = (w1c * s).astype(np.float32)
    scl = np.zeros((128, AH_ * 200), np.float32)  # expanded: block c = scale col
    for c in range(AH_):
        scl[0:64, c * 200 : (c + 1) * 200] = w1cs[:, c : c + 1]
        scl[64:128, c * 200 : (c + 1) * 200] = w1cs[:, AH_ + c : AH_ + c + 1]
    scl = scl.astype(BF)
    # compose through the layernorm: aT' = w1as^T @ [hT; 1] = waComp^T @ [xT; 1]
    w1a_s, w1b_s = w1a * s, w1b * s
    w1as = np.zeros((D + 1, A + 1), np.float32)
    w1as[0:D, 0:A] = ln_w @ w1a_s
    w1as[D, 0:A] = ln_b @ w1a_s
    w1as[D, A] = 1.0  # ones output col (reads xT's ones row)
    w1as = w1as.astype(BF)
    w1bs = np.zeros((D + 1, A), np.float32)
    w1bs[0:D] = ln_w @ w1b_s
    w1bs[D] = ln_b @ w1b_s + b1p * s  # b1 folded in
    w1bs = w1bs.astype(BF)
    lnw = np.vstack([ln_w, ln_b[None, :]]).astype(BF)  # [D+1, H]
    # one-hot tile [128, AH*L]: row c selects aT' row c (top); row 64+AH+c
    # selects row AH+c (bottom); rows 36/100 are the per-batch c-row targets
    oh = np.zeros((128, AH * L), dtype=np.float32)
    for c in range(AH):
        oh[c, c * L : (c + 1) * L] = 1.0
        oh[64 + AH + c, c * L : (c + 1) * L] = 1.0
    oh = oh.astype(BF)
    idm = np.eye(128, dtype=BF)
    m0 = (np.arange(L)[None, :] >= np.arange(J0)[:, None]).astype(BF)
    m1 = (np.arange(J1)[None, :] >= np.arange(J1)[:, None]).astype(BF)
    return (
        dict(scl=scl, w1as=w1as, w1bs=w1bs, lnw=lnw, oh=oh, idm=idm, m0=m0, m1=m1),
        npos,
        float(b2),
    )


def _build(npos, b2):
    import concourse.bacc as bacc
    import concourse.tile as tile
    from concourse import mybir

    f32, bf16 = mybir.dt.float32, mybir.dt.bfloat16
    LR = mybir.ActivationFunctionType.Lrelu
    lrelu1 = _get_lrelu1()

    nc = bacc.Bacc("TRN2", target_bir_lowering=False, debug=False)
    x_d = nc.dram_tensor("x", [BPC, L, D], bf16, kind="ExternalInput")
    out_d = nc.dram_tensor("out", [BPC, L, H], f32, kind="ExternalOutput")
    scl_d = nc.dram_tensor("scl", [128, AH * L], bf16, kind="ExternalInput")
    w1as_d = nc.dram_tensor("w1as", [D + 1, A + 1], bf16, kind="ExternalInput")
    w1bs_d = nc.dram_tensor("w1bs", [D + 1, A], bf16, kind="ExternalInput")
    lnw_d = nc.dram_tensor("lnw", [D + 1, H], bf16, kind="ExternalInput")
    oh_d = nc.dram_tensor("oh", [128, AH * L], bf16, kind="ExternalInput")
    idm_d = nc.dram_tensor("idm", [128, 128], bf16, kind="ExternalInput")
    m0_d = nc.dram_tensor("m0", [J0, L], bf16, kind="ExternalInput")
    m1_d = nc.dram_tensor("m1", [J1, J1], bf16, kind="ExternalInput")


    with tile.TileContext(nc) as tc:
        with (
            tc.tile_pool(name="consts", bufs=1) as cp,
            tc.tile_pool(name="prep", bufs=1) as pp,
            tc.tile_pool(name="work", bufs=2) as wp,
            tc.tile_pool(name="psw", bufs=3, space="PSUM") as psw,
            tc.tile_pool(name="psp", bufs=2, space="PSUM") as psp,
        ):
            idm = cp.tile([128, 128], bf16)
            nc.sync.dma_start(idm[:], idm_d[:])
            lnw = cp.tile([D + 1, H], bf16)
            nc.sync.dma_start(lnw[:], lnw_d[:])
            w1as = cp.tile([D + 1, A + 1], bf16)
            nc.sync.dma_start(w1as[:], w1as_d[:])
            w1bs = cp.tile([D + 1, A], bf16)
            nc.sync.dma_start(w1bs[:], w1bs_d[:])
            scl = cp.tile([128, AH * L], bf16)
            nc.scalar.dma_start(scl[:], scl_d[:])
            m0 = cp.tile([J0, L], bf16)
            nc.scalar.dma_start(m0[:], m0_d[:])
            m1 = cp.tile([J1, J1], bf16)
            nc.scalar.dma_start(m1[:], m1_d[:])
            # per-batch one-hot tiles: crow rows 36/100 rewritten per batch
            OH2 = []
            for k in range(BPC):
                t = cp.tile([128, AH * L], bf16, tag=f"oh2_{k}")
                nc.scalar.dma_start(t[:], oh_d[:])
                OH2.append(t)

            def build_rhs(hh_):
                rhs = wp.tile([128, AH * L], bf16, tag="rhs")
                hv = (
                    hh_[:, :]
                    .rearrange("p (o x) -> p o x", o=1)
                    .broadcast_to([128, AH, L])
                )
                sv = scl[:, :].rearrange("p (c x) -> p c x", x=L)
                rv = rhs[:, :].rearrange("p (c x) -> p c x", x=L)
                nc.vector.tensor_mul(rv[:], hv[:], sv[:])
                return rhs

            # ---------- phase 1: per-batch prep ----------
            HH, ATS, H0, H1 = [], [], [], []
            RHS = {}
            for bi in range(BPC):
                x0 = wp.tile([128, D], bf16, tag="x0")
                nc.sync.dma_start(x0[:], x_d[bi, 0:128, :])
                x1 = wp.tile([J1, D], bf16, tag="x1")
                nc.sync.dma_start(x1[:], x_d[bi, 128:L, :])
                xT = wp.tile([D + 1, L], bf16, tag="xT")
                pt0 = psp.tile([D, 128], bf16, tag="pp")
                nc.tensor.transpose(pt0[:], x0[:], idm[:, :])
                nc.vector.tensor_copy(xT[0:D, 0:128], pt0[:])
                pt1 = psp.tile([D, J1], bf16, tag="pp")
                nc.tensor.transpose(pt1[:], x1[:], idm[0:J1, 0:J1])
                nc.vector.tensor_copy(xT[0:D, 128:L], pt1[:])
                nc.vector.memset(xT[D : D + 1, :], 1.0)

                ph = psp.tile([H, L], f32, tag="pp")
                nc.tensor.matmul(ph[:], lnw[:], xT[:], start=True, stop=True)
                hh = pp.tile([128, L], bf16, tag=f"hh{bi}")  # [hT; hT]
                nc.vector.tensor_copy(hh[0:H, :], ph[:])
                nc.vector.tensor_copy(hh[H:128, :], ph[:])

                ph0 = psp.tile([128, H], f32, tag="pp")
                nc.tensor.matmul(ph0[:], xT[:, 0:128], lnw[:], start=True, stop=True)
                h0 = pp.tile([128, H], bf16, tag=f"h0{bi}")
                nc.vector.tensor_copy(h0[:], ph0[:])
                ph1 = psp.tile([J1, H], f32, tag="pp")
                nc.tensor.matmul(ph1[:], xT[:, 128:L], lnw[:], start=True, stop=True)
                h1 = pp.tile([J1, H], bf16, tag=f"h1{bi}")
                nc.vector.tensor_copy(h1[:], ph1[:])

                # aTs: rows 0-36 = [aT'; ones], rows 64-100 = same (bottom copy)
                pa = psp.tile([A + 1, L], f32, tag="pp")
                nc.tensor.matmul(pa[:], w1as[:], xT[:], start=True, stop=True)
                aTs = pp.tile([128, L], bf16, tag=f"aTs{bi}")
                nc.vector.tensor_copy(aTs[0 : A + 1, :], pa[:])
                nc.vector.tensor_copy(aTs[64 : 64 + A + 1, :], pa[:])

                # cT' (+b1) -> flatten into one-hot tile rows 36 / 100
                pc = psp.tile([A, L], f32, tag="pp")
                nc.tensor.matmul(pc[:], w1bs[:], xT[:], start=True, stop=True)
                ctb = wp.tile([A, L], bf16, tag="ctb")
                nc.vector.tensor_copy(ctb[:], pc[:])
                oh2 = OH2[bi]
                nc.gpsimd.dma_start(
                    oh2[A : A + 1, :].rearrange("p (c x) -> p c x", x=L)[0:1],
                    ctb[0:AH, :],
                )
                nc.gpsimd.dma_start(
                    oh2[64 + A : 64 + A + 1, :].rearrange("p (c x) -> p c x", x=L)[0:1],
                    ctb[AH:A, :],
                )

                HH.append(hh)
                ATS.append(aTs)
                H0.append(h0)
                H1.append(h1)
                if bi < 2:
                    RHS[bi] = build_rhs(hh)

            # ---------- phase 2: packed channel waves, folds, output ----------
            for bi in range(BPC):
                hh, aTs = HH[bi], ATS[bi]
                h0, h1 = H0[bi], H1[bi]
                oh2 = OH2[bi]
                rhs = RHS.pop(bi)

                r0 = wp.tile([J0, A * L], bf16, tag="r0")
                r1 = wp.tile([J1, A * J1], bf16, tag="r1")

                # jb0: 9 waves; wave t = top pair (2t,2t+1) + bottom pair (+18)
                for t in range(AH // 2):
                    pw = psw.tile([J0, 1024], f32, tag="pw")
                    cols = slice(2 * t * L, (2 * t + 2) * L)
                    nc.tensor.matmul(
                        pw[:, 0:400], hh[0:H, 0:J0], rhs[0:H, cols],
                        start=True, stop=False,
                    )
                    nc.tensor.matmul(
                        pw[:, 0:400], aTs[0 : A + 1, 0:J0], oh2[0 : A + 1, cols],
                        start=False, stop=True,
                    )
                    nc.tensor.matmul(
                        pw[:, 512:912], hh[H:128, 0:J0], rhs[H:128, cols],
                        start=True, stop=False,
                    )
                    nc.tensor.matmul(
                        pw[:, 512:912],
                        aTs[64 : 64 + A + 1, 0:J0],
                        oh2[64 : 64 + A + 1, cols],
                        start=False, stop=True,
                    )
                    r0v = r0[:, :].rearrange("p (g y) -> p g y", y=AH * L)[
                        :, :, 2 * t * L : (2 * t + 2) * L
                    ]
                    pwv = pw[:, :].rearrange("p (g y) -> p g y", y=512)[:, :, 0:400]
                    if t in JB0_V_WAVES:
                        nc.vector._custom_dve(
                            lrelu1, out=r0v, in0=pwv, s0=NEG_SLOPE
                        )
                    else:
                        nc.scalar.activation(r0v, pwv, LR, alpha=NEG_SLOPE)

                # jb1: 5 waves of up-to-4 channels per half, 128-padded slots
                rhv_t = rhs[0:H, :].rearrange("p (c x) -> p c x", x=L)
                rhv_b = rhs[H:128, :].rearrange("p (c x) -> p c x", x=L)
                ohv_t = oh2[0 : A + 1, :].rearrange("p (c x) -> p c x", x=L)
                ohv_b = oh2[64 : 64 + A + 1, :].rearrange("p (c x) -> p c x", x=L)
                for w in range((AH + 3) // 4):
                    c0 = 4 * w
                    gn = min(4, AH - c0)
                    pz = psw.tile([J1, 1024], f32, tag="pw")
                    pzv = pz[:, :].rearrange("p (g x) -> p g x", x=128)
                    nc.tensor.matmul(
                        pzv[:, 0:gn, 0:J1],
                        hh[0:H, 128:L],
                        rhv_t[:, c0 : c0 + gn, 128:L],
                        start=True, stop=False,
                    )
                    nc.tensor.matmul(
                        pzv[:, 0:gn, 0:J1],
                        aTs[0 : A + 1, 128:L],
                        ohv_t[:, c0 : c0 + gn, 128:L],
                        start=False, stop=True,
                    )
                    nc.tensor.matmul(
                        pzv[:, 4 : 4 + gn, 0:J1],
                        hh[H:128, 128:L],
                        rhv_b[:, c0 : c0 + gn, 128:L],
                        start=True, stop=False,
                    )
                    nc.tensor.matmul(
                        pzv[:, 4 : 4 + gn, 0:J1],
                        aTs[64 : 64 + A + 1, 128:L],
                        ohv_b[:, c0 : c0 + gn, 128:L],
                        start=False, stop=True,
                    )
                    if w in JB1_V_WAVES:
                        for g in range(2):
                            nc.vector._custom_dve(
                                lrelu1,
                                out=r1[:, :].rearrange(
                                    "p (g c x) -> p g c x", g=2, x=J1
                                )[:, g, c0 : c0 + gn, :],
                                in0=pz[:, :].rearrange(
                                    "p (g s x) -> p g s x", g=2, x=128
                                )[:, g, 0:gn, 0:J1],
                                s0=NEG_SLOPE,
                            )
                    else:
                        nc.scalar.activation(
                            r1[:, :]
                            .rearrange("p (g c x) -> p g c x", g=2, x=J1)[
                                :, :, c0 : c0 + gn, :
                            ],
                            pz[:, :]
                            .rearrange("p (g s x) -> p g s x", g=2, x=128)[
                                :, :, 0:gn, 0:J1
                            ],
                            LR,
                            alpha=NEG_SLOPE,
                        )

                if bi + 2 < BPC:
                    RHS[bi + 2] = build_rhs(HH[bi + 2])

                # signed fold: L1 = one big (pos - neg) pass over min(P,N)
                # pairs, leftover blocks merged in chunks, then add-tree;
                # final pass fuses +b2 and causal mask in one STT.
                P, N = npos, A - npos
                m = min(P, N)

                sm0 = wp.tile([J0, L], bf16, tag="sm0")
                sm1 = wp.tile([J1, J1], bf16, tag="sm1")
                for reg, stride, sm, msk in ((r0, L, sm0, m0), (r1, J1, sm1, m1)):
                    if m > 0:
                        nc.vector.tensor_sub(
                            reg[:, 0 : m * stride],
                            reg[:, 0 : m * stride],
                            reg[:, P * stride : (P + m) * stride],
                        )
                        # leftovers: extra pos at [m:P) (add) or extra neg
                        # at [P+m:A) (subtract), merged into d-blocks
                        if P > N:
                            off, sign = N, mybir.AluOpType.add
                        else:
                            off, sign = P + m, mybir.AluOpType.subtract
                        extra = max(P, N) - m
                        o = off
                        while extra > 0:
                            w_ = min(m, extra)
                            nc.vector.tensor_tensor(
                                reg[:, 0 : w_ * stride],
                                reg[:, 0 : w_ * stride],
                                reg[:, o * stride : (o + w_) * stride],
                                sign,
                            )
                            o += w_
                            extra -= w_
                        W = m
                    else:
                        W = A  # all one sign; tree over everything
                    while W > 1:
                        half = W // 2
                        keep = W - half
                        nc.vector.tensor_add(
                            reg[:, 0 : half * stride],
                            reg[:, 0 : half * stride],
                            reg[:, keep * stride : W * stride],
                        )
                        W = keep
                    if m > 0 or P > 0:
                        nc.vector.scalar_tensor_tensor(
                            sm[:],
                            reg[:, 0:stride],
                            b2,
                            msk[:],
                            mybir.AluOpType.add,
                            mybir.AluOpType.mult,
                        )
                    else:  # all-negative: negate, +b2, then mask
                        nc.vector.tensor_scalar(
                            sm[:], reg[:, 0:stride], -1.0, b2,
                            mybir.AluOpType.mult, mybir.AluOpType.add,
                        )
                        nc.vector.tensor_mul(sm[:], sm[:], msk[:])

                # out = masked-score^T @ h
                po1 = psp.tile([128, H], f32, tag="pp")
                nc.tensor.matmul(po1[:], sm0[:, 0:128], h0[:], start=True, stop=True)
                po2 = psp.tile([J1, H], f32, tag="pp")
                nc.tensor.matmul(po2[:], sm0[:, 128:L], h0[:], start=True, stop=False)
                nc.tensor.matmul(po2[:], sm1[:], h1[:], start=False, stop=True)
                o0 = wp.tile([128, H], f32, tag="o0")
                nc.vector.tensor_copy(o0[:], po1[:])
                o1 = wp.tile([J1, H], f32, tag="o1")
                nc.vector.tensor_copy(o1[:], po2[:])
                nc.sync.dma_start(out_d[bi, 0:128, :], o0[:])
                nc.sync.dma_start(out_d[bi, 128:L, :], o1[:])

    if not nc.is_finalized():
        nc.finalize()
    return nc


_CACHE = {}


def kernel(x, ln_w, ln_b, w1, b1, w2, b2):
    from concourse.bass_utils import run_bass_kernel_spmd

    x = np.asarray(x, dtype=np.float32)
    consts, npos, b2f = _host_prep(
        np.asarray(ln_w, np.float32),
        np.asarray(ln_b, np.float32),
        np.asarray(w1, np.float32),
        np.asarray(b1, np.float32),
        np.asarray(w2, np.float32),
        np.asarray(b2, np.float32),
    )
    key = (npos, round(b2f, 9))
    if key not in _CACHE:
        _CACHE[key] = _build(npos, b2f)
    nc = _CACHE[key]

    xb = x.astype(BF)
    in_maps = []
    for c in range(NCORES):
        m = {"x": xb[c * BPC : (c + 1) * BPC]}
        m.update(consts)
        in_maps.append(m)

    trace = bool(int(os.environ.get("KERNEL_TRACE", "0")))
    res = run_bass_kernel_spmd(nc, in_maps, list(range(NCORES)), trace=trace)
    out = np.concatenate([res.results[c]["out"] for c in range(NCORES)], axis=0)
    if trace:
        kernel.last_exec_time_ns = res.exec_time_ns
        kernel.last_results = res
    return out.astype(np.float32)

